# revision 6
# baseline (speedup 1.0000x reference)
"""Trainium2 Bass kernel for AlternateCrossAttentionBlock.

Math trick: the (b,w,h,s,c) tensors k,v are never materialized.
  attn_logits[c,d] = [S*vx[c]*kx[d] + vx[c]*Ks_sum[d] + Vs_sum[c]*kx[d] + M[c,d]]/8
with M = vs^T @ ks (64x64 per batch), Ks_sum/Vs_sum the s-sums of ks/vs.

Sharding: pure data parallel, core i -> batch i//2, w-half i%2 (512 positions).
No collectives needed.
"""

import numpy as np

import concourse.bass as bass
import concourse.bacc as bacc
import concourse.mybir as mybir
import concourse.tile as tile
from concourse.masks import make_identity
from concourse.bass_utils import run_bass_kernel_spmd

F32 = mybir.dt.float32
ALU = mybir.AluOpType
ACTF = mybir.ActivationFunctionType

P = 128     # partitions
NPOS = 512  # positions per core (16 w * 32 h)
NCH = 64    # attention inner dim
C = 128     # x channels
S = 256     # text seq len
D = 512     # text embed dim
NCHUNK = NPOS // P  # 4 chunks of 128 positions


def build_nc():
    nc = bacc.Bacc("TRN2", target_bir_lowering=False, debug=False)

    # --- DRAM parameters (per-core shards; weights replicated) ---
    x_d = nc.declare_dram_parameter("x", [NPOS, C], F32, isOutput=False)
    s_d = nc.declare_dram_parameter("s", [S, D], F32, isOutput=False)
    a_d = nc.declare_dram_parameter("a", [S, 1], F32, isOutput=False)
    wqkv_d = nc.declare_dram_parameter("Wqkv", [C, 3 * NCH], F32, isOutput=False)
    bqkv_d = nc.declare_dram_parameter("bqkv", [1, 3 * NCH], F32, isOutput=False)
    wks_d = nc.declare_dram_parameter("Wks", [D, NCH], F32, isOutput=False)
    wvs_d = nc.declare_dram_parameter("Wvs", [D, NCH], F32, isOutput=False)
    bks_d = nc.declare_dram_parameter("bks", [1, NCH], F32, isOutput=False)
    bvs_d = nc.declare_dram_parameter("bvs", [1, NCH], F32, isOutput=False)
    wout_d = nc.declare_dram_parameter("Wout", [NCH, C], F32, isOutput=False)
    bout_d = nc.declare_dram_parameter("bout", [1, C], F32, isOutput=False)
    lns_d = nc.declare_dram_parameter("ln_scale", [1, C], F32, isOutput=False)
    lnb_d = nc.declare_dram_parameter("ln_bias", [1, C], F32, isOutput=False)
    out_d = nc.declare_dram_parameter("out", [NPOS, C], F32, isOutput=True)

    # DRAM scratch for partition-broadcast bounces
    scr_m8 = nc.dram_tensor("scr_m8", [NCH, NCH], F32)
    scr_ks = nc.dram_tensor("scr_ks", [1, NCH], F32)
    scr_vs = nc.dram_tensor("scr_vs", [1, NCH], F32)

    with tile.TileContext(nc) as tc:
        with (
            tc.tile_pool(name="consts", bufs=1) as consts,
            tc.tile_pool(name="sbuf", bufs=2) as pool,
            tc.tile_pool(name="big", bufs=2) as bigpool,
            tc.tile_pool(name="psum", bufs=2, space="PSUM") as psum,
        ):
            # ---------- constants ----------
            ident = consts.tile([P, P], F32)
            make_identity(nc, ident)
            ones_col = consts.tile([P, 1], F32)
            nc.gpsimd.memset(ones_col, 1.0)
            eps_col = consts.tile([P, 1], F32)
            nc.gpsimd.memset(eps_col, 1e-6)

            # replicated weights into SBUF
            wqkv = consts.tile([C, 3 * NCH], F32)
            nc.sync.dma_start(wqkv, wqkv_d[:])
            bqkv_bc = consts.tile([P, 3 * NCH], F32)
            nc.sync.dma_start(bqkv_bc, bqkv_d.broadcast_to([P, 3 * NCH]))
            wks = consts.tile([P, 4, NCH], F32)
            nc.sync.dma_start(wks, wks_d.rearrange("(j p) n -> p j n", p=P))
            wvs = consts.tile([P, 4, NCH], F32)
            nc.sync.dma_start(wvs, wvs_d.rearrange("(j p) n -> p j n", p=P))
            bks_bc = consts.tile([P, NCH], F32)
            nc.sync.dma_start(bks_bc, bks_d.broadcast_to([P, NCH]))
            bvs_bc = consts.tile([P, NCH], F32)
            nc.sync.dma_start(bvs_bc, bvs_d.broadcast_to([P, NCH]))
            wout = consts.tile([NCH, C], F32)
            nc.sync.dma_start(wout, wout_d[:])
            bout_bc = consts.tile([P, C], F32)
            nc.sync.dma_start(bout_bc, bout_d.broadcast_to([P, C]))
            lns_bc = consts.tile([P, C], F32)
            nc.sync.dma_start(lns_bc, lns_d.broadcast_to([P, C]))
            lnb_bc = consts.tile([P, C], F32)
            nc.sync.dma_start(lnb_bc, lnb_d.broadcast_to([P, C]))

            # ---------- s-side: te = s*a, ks/vs, M, Ks_sum, Vs_sum ----------
            te = []
            for i in range(2):
                s_t = pool.tile([P, D], F32, tag="s_t")
                nc.sync.dma_start(s_t, s_d[i * P:(i + 1) * P, :])
                a_t = pool.tile([P, 1], F32, tag="a_t")
                nc.sync.dma_start(a_t, a_d[i * P:(i + 1) * P, :])
                te_t = consts.tile([P, D], F32, tag=f"te{i}")
                nc.vector.tensor_scalar_mul(te_t, s_t, a_t)
                te.append(te_t)

            # transpose te -> teT_j tiles (128 d x 256 s), j = d-block
            teT = []
            for j in range(4):
                teT_j = consts.tile([P, S], F32, tag=f"teT{j}")
                teT.append(teT_j)
            for i in range(2):
                for j in range(4):
                    pt = psum.tile([P, P], F32, tag="tr")
                    nc.tensor.transpose(pt, te[i][:, j * P:(j + 1) * P], ident)
                    nc.vector.tensor_copy(teT[j][:, i * P:(i + 1) * P], pt)

            # ks/vs chunks (s-chunk m on partitions), with bias
            ks_sb, vs_sb = [], []
            for m in range(2):
                pks = psum.tile([P, NCH], F32, tag="mm")
                for j in range(4):
                    nc.tensor.matmul(pks, teT[j][:, m * P:(m + 1) * P], wks[:, j, :],
                                     start=(j == 0), stop=(j == 3))
                ks_m = consts.tile([P, NCH], F32, tag=f"ks{m}")
                nc.vector.tensor_tensor(ks_m, pks, bks_bc, op=ALU.add)
                pvs = psum.tile([P, NCH], F32, tag="mm")
                for j in range(4):
                    nc.tensor.matmul(pvs, teT[j][:, m * P:(m + 1) * P], wvs[:, j, :],
                                     start=(j == 0), stop=(j == 3))
                vs_m = consts.tile([P, NCH], F32, tag=f"vs{m}")
                nc.vector.tensor_tensor(vs_m, pvs, bvs_bc, op=ALU.add)
                ks_sb.append(ks_m)
                vs_sb.append(vs_m)

            # M = vs^T @ ks  (64x64), scaled by 1/8
            pM = psum.tile([NCH, NCH], F32, tag="mm")
            for m in range(2):
                nc.tensor.matmul(pM, vs_sb[m], ks_sb[m], start=(m == 0), stop=(m == 1))
            m8 = pool.tile([NCH, NCH], F32, tag="m8")
            nc.scalar.mul(m8, pM, 0.125)
            nc.sync.dma_start(scr_m8[:], m8)

            # Ks_sum, Vs_sum rows (1x64), scaled by 1/8
            pKs = psum.tile([1, NCH], F32, tag="mm")
            pVs = psum.tile([1, NCH], F32, tag="tr")
            for m in range(2):
                nc.tensor.matmul(pKs, ones_col, ks_sb[m], start=(m == 0), stop=(m == 1))
            for m in range(2):
                nc.tensor.matmul(pVs, ones_col, vs_sb[m], start=(m == 0), stop=(m == 1))
            ks8_row = pool.tile([1, NCH], F32, tag="ks8r")
            vs8_row = pool.tile([1, NCH], F32, tag="vs8r")
            nc.scalar.mul(ks8_row, pKs, 0.125)
            nc.scalar.mul(vs8_row, pVs, 0.125)
            nc.sync.dma_start(scr_ks[:], ks8_row)
            nc.sync.dma_start(scr_vs[:], vs8_row)

            # broadcast across partitions via DRAM bounce
            m8_bc = consts.tile([P, NCH, NCH], F32)
            nc.sync.dma_start(m8_bc, scr_m8[None].broadcast_to([P, NCH, NCH]))
            ks8_bc = consts.tile([P, NCH], F32)
            nc.sync.dma_start(ks8_bc, scr_ks.broadcast_to([P, NCH]))
            vs8_bc = consts.tile([P, NCH], F32)
            nc.sync.dma_start(vs8_bc, scr_vs.broadcast_to([P, NCH]))

            # ---------- x-side: per chunk of 128 positions ----------
            for t in range(NCHUNK):
                xt = pool.tile([P, C], F32, tag="xt")
                nc.sync.dma_start(xt, x_d[t * P:(t + 1) * P, :])
                pxT = psum.tile([P, P], F32, tag="tr")
                nc.tensor.transpose(pxT, xt, ident)
                xT = pool.tile([P, P], F32, tag="xT")
                nc.vector.tensor_copy(xT, pxT)

                pqkv = psum.tile([P, 3 * NCH], F32, tag="mm")
                nc.tensor.matmul(pqkv, xT, wqkv, start=True, stop=True)
                qkv = pool.tile([P, 3 * NCH], F32, tag="qkv")
                nc.vector.tensor_tensor(qkv, pqkv, bqkv_bc, op=ALU.add)
                q_ap = qkv[:, 0:NCH]
                kx_ap = qkv[:, NCH:2 * NCH]
                vx_ap = qkv[:, 2 * NCH:3 * NCH]

                # a_vec = 32*kx + Ks_sum/8
                a_vec = pool.tile([P, NCH], F32, tag="a_vec")
                nc.vector.scalar_tensor_tensor(a_vec, kx_ap, 32.0, ks8_bc,
                                               op0=ALU.mult, op1=ALU.add)

                # L[p,j,k] = vx[p,j]*a_vec[p,k] + vs8[j]*kx[p,k] + m8[j,k]
                L = bigpool.tile([P, NCH, NCH], F32, tag="L")
                tmp = bigpool.tile([P, NCH, NCH], F32, tag="tmp")
                u_view = vx_ap[:, :, None].broadcast_to([P, NCH, NCH])
                av_view = a_vec[:, None, :].broadcast_to([P, NCH, NCH])
                b_view = vs8_bc[:, :, None].broadcast_to([P, NCH, NCH])
                c_view = kx_ap[:, None, :].broadcast_to([P, NCH, NCH])
                nc.vector.tensor_tensor(L, u_view, av_view, op=ALU.mult)
                nc.vector.tensor_tensor(tmp, b_view, c_view, op=ALU.mult)
                nc.vector.tensor_tensor(L, L, tmp, op=ALU.add)
                nc.vector.tensor_tensor(L, L, m8_bc, op=ALU.add)

                # softmax over k (innermost)
                R = pool.tile([P, NCH], F32, tag="R")
                nc.vector.tensor_reduce(R, L, axis=mybir.AxisListType.X, op=ALU.max)
                nc.vector.tensor_tensor(L, L, R[:, :, None].broadcast_to([P, NCH, NCH]),
                                        op=ALU.subtract)
                E = bigpool.tile([P, NCH, NCH], F32, tag="E")
                nc.scalar.activation(E, L, ACTF.Exp)
                Z = pool.tile([P, NCH], F32, tag="Z")
                nc.vector.tensor_reduce(Z, E, axis=mybir.AxisListType.X, op=ALU.add)

                # w = q / Z
                Zr = pool.tile([P, NCH], F32, tag="Zr")
                nc.vector.reciprocal(Zr, Z)
                w = pool.tile([P, NCH], F32, tag="w")
                nc.vector.tensor_tensor(w, q_ap, Zr, op=ALU.mult)

                # O[p,k] = sum_j w[p,j] * E[p,j,k]
                nc.vector.tensor_tensor(E, E, w[:, :, None].broadcast_to([P, NCH, NCH]),
                                        op=ALU.mult)
                O = pool.tile([P, NCH], F32, tag="O")
                nc.vector.tensor_reduce(O, E.rearrange("p j k -> p k j"),
                                        axis=mybir.AxisListType.X, op=ALU.add)

                # out projection + residual + layernorm
                pOT = psum.tile([NCH, P], F32, tag="tr")
                nc.tensor.transpose(pOT, O, ident)
                OT = pool.tile([NCH, P], F32, tag="OT")
                nc.vector.tensor_copy(OT, pOT)
                py = psum.tile([P, C], F32, tag="mm")
                nc.tensor.matmul(py, OT, wout, start=True, stop=True)

                y = pool.tile([P, C], F32, tag="y")
                nc.vector.tensor_tensor(y, py, bout_bc, op=ALU.add)
                nc.vector.tensor_tensor(y, y, xt, op=ALU.add)

                mu = pool.tile([P, 1], F32, tag="mu")
                nc.vector.tensor_reduce(mu, y, axis=mybir.AxisListType.X, op=ALU.add)
                nc.scalar.mul(mu, mu, 1.0 / C)
                xc = pool.tile([P, C], F32, tag="xc")
                nc.vector.tensor_scalar_sub(xc, y, mu)
                sqd = pool.tile([P, C], F32, tag="sqd")
                ssq = pool.tile([P, 1], F32, tag="ssq")
                nc.scalar.activation(sqd, xc, ACTF.Square, accum_out=ssq)
                sd = pool.tile([P, 1], F32, tag="sd")
                nc.scalar.activation(sd, ssq, ACTF.Sqrt, bias=eps_col, scale=1.0 / C)
                rstd = pool.tile([P, 1], F32, tag="rstd")
                nc.vector.reciprocal(rstd, sd)

                yo = pool.tile([P, C], F32, tag="yo")
                nc.vector.scalar_tensor_tensor(yo, xc, rstd, lns_bc,
                                               op0=ALU.mult, op1=ALU.mult)
                nc.vector.tensor_tensor(yo, yo, lnb_bc, op=ALU.add)
                nc.sync.dma_start(out_d[t * P:(t + 1) * P, :], yo)

    nc.compile()
    return nc


_NC = None


def kernel(**inputs):
    global _NC
    x = np.asarray(inputs["x"], np.float32)    # (4,32,32,128)
    s = np.asarray(inputs["s"], np.float32)    # (4,256,512)
    a = np.asarray(inputs["a"], np.float32)    # (4,256)
    Wq, bq = np.asarray(inputs["Wq"], np.float32), np.asarray(inputs["bq"], np.float32)
    Wkx, bkx = np.asarray(inputs["Wkx"], np.float32), np.asarray(inputs["bkx"], np.float32)
    Wvx, bvx = np.asarray(inputs["Wvx"], np.float32), np.asarray(inputs["bvx"], np.float32)
    Wks, bks = np.asarray(inputs["Wks"], np.float32), np.asarray(inputs["bks"], np.float32)
    Wvs, bvs = np.asarray(inputs["Wvs"], np.float32), np.asarray(inputs["bvs"], np.float32)
    Wout, bout = np.asarray(inputs["Wout"], np.float32), np.asarray(inputs["bout"], np.float32)
    lns, lnb = np.asarray(inputs["ln_scale"], np.float32), np.asarray(inputs["ln_bias"], np.float32)

    B, W, H, Cc = x.shape
    Wqkv = np.concatenate([Wq, Wkx, Wvx], axis=1)          # (128,192)
    bqkv = np.concatenate([bq, bkx, bvx])[None, :]         # (1,192)

    if _NC is None:
        _NC = build_nc()
    nc = _NC

    in_maps = []
    for i in range(8):
        b, wh = i // 2, i % 2
        in_maps.append({
            "x": np.ascontiguousarray(x[b, wh * 16:(wh + 1) * 16].reshape(NPOS, C)),
            "s": np.ascontiguousarray(s[b]),
            "a": np.ascontiguousarray(a[b].reshape(S, 1)),
            "Wqkv": Wqkv, "bqkv": bqkv,
            "Wks": Wks, "Wvs": Wvs,
            "bks": bks[None, :], "bvs": bvs[None, :],
            "Wout": Wout, "bout": bout[None, :],
            "ln_scale": lns[None, :], "ln_bias": lnb[None, :],
        })

    res = run_bass_kernel_spmd(nc, in_maps, core_ids=list(range(8)))
    y = np.empty((B, W, H, Cc), np.float32)
    for i in range(8):
        b, wh = i // 2, i % 2
        y[b, wh * 16:(wh + 1) * 16] = res.results[i]["out"].reshape(16, H, Cc)
    return y
